# revision 20
# baseline (speedup 1.0000x reference)
"""Trainium2 Bass kernel for nn_MultiHeadAttention (B=4, S=2048, D=1024, H=16).

Sharding: 8 cores = 4 batches x 2 head-groups. Core c handles batch c//2 and
heads (c%2)*8 .. (c%2)*8+7. Each core computes Q/K/V projections for its 512
columns, causal attention for its 8 heads, and two partial output
projections (row-parallel over wo, split f={0,1} / f={2,3}). Host sums the
partials per batch and adds bo.

Per-core layout: x^T [D, S] shipped pre-transposed (bf16). Q^T/K^T per
head-pair packed [128, S] float32r (head h=2hp+rr in rows 64*rr..64*rr+63;
scores contract over head_dim with base-partition-64 APs for the odd head —
the f32r path supports this). V natural [S, 65] per head with a trailing
ones column so P^T@V also produces softmax row-sums. S^T = K^T.T @ Q^T per
128-k-tile into [128, 1024] PSUM windows, exp on ACT, O^T accumulates in
[65, 1024] PSUM chunks (row 64 = sums).

Scheduling: the two heads of a pair interleave step-by-step (rr alternates
per ki) and each step's PV matmul is emitted one step late so the PE never
waits on the exp it just issued. Scores run in f32r (2x PE cycles) which
keeps the tensor engine saturated during ACT-paced stretches — the PE HAM
clock gate re-throttles to 1.2 GHz whenever PE duty drops, which costs more
than the extra f32r cycles. All deferrable GEMM work (V tiles 8-15, the
next head-pair's Q/K projection, finished out01/out23 chunks) is spread
evenly into the attention windows as PE filler. Normalization evacuates
O^T + sums to SBUF, reshapes sums via small DRAM round-trips for a cheap
[128, 8] reciprocal, and multiplies on the otherwise-idle GpSimd engine.
The odd (rr=1) head lands in stg rows 0-63 by a direct write (wo rows are
swapped host-side to match) since it finishes last.
"""

import numpy as np

import concourse.bass as bass
import concourse.mybir as mybir
import concourse.tile as tile
from concourse import bacc
from concourse.masks import make_upper_triangular

F32 = mybir.dt.float32
F32R = mybir.dt.float32r
BF16 = mybir.dt.bfloat16
P = 128


def build_nc(S=2048, D=1024, HN=8, HD=64):
    """Per-core Bass module. HN = local heads, C = HN*HD local cols."""
    MD = BF16
    C = HN * HD
    NT = S // P        # token tiles
    ND = D // P        # d tiles (contraction for projections)
    NM = C // P        # head-pairs (2 heads of HD=64 per 128-partition tile)
    QW = 1024          # O^T psum chunk width (two PSUM banks; chunk == window)
    SW = 1024          # scores window width (two PSUM banks)
    W5 = 512           # projection N-chunk width
    WD = 512           # phase-D N-chunk width
    NCH = S // SW      # score windows per head
    NQC = S // QW      # O^T chunks per head
    SQF = QW // P      # free size of the [P, SQF] reciprocal reshape
    SCALE = 1.0 / float(np.sqrt(HD))
    VW = HD + 1        # V columns per head incl. trailing ones column
    CV = HN * VW       # augmented V cols

    nc = bacc.Bacc("TRN2", target_bir_lowering=False)

    xT_d = nc.dram_tensor("xT", [D, S], MD, kind="ExternalInput")
    wq_d = nc.dram_tensor("wq", [D, C], MD, kind="ExternalInput")
    wk_d = nc.dram_tensor("wk", [D, C], MD, kind="ExternalInput")
    wv_d = nc.dram_tensor("wv", [D, CV], MD, kind="ExternalInput")
    wo_d = nc.dram_tensor("wo", [C, D], MD, kind="ExternalInput")
    bq_d = nc.dram_tensor("bq", [C], F32, kind="ExternalInput")
    bk_d = nc.dram_tensor("bk", [C], F32, kind="ExternalInput")
    bv_d = nc.dram_tensor("bv", [CV], F32, kind="ExternalInput")
    # out01 = heads pairs 0-1 partial, out23 = head pairs 2-3; host adds.
    out01_d = nc.dram_tensor("out01", [S, D], F32, kind="ExternalOutput")
    out23_d = nc.dram_tensor("out23", [S, D], F32, kind="ExternalOutput")

    with tile.TileContext(nc) as tc:
        from contextlib import ExitStack

        with ExitStack() as ctx:
            singles = ctx.enter_context(tc.tile_pool(name="singles", bufs=1))
            # ut1[k, q] = 1.0 where k <= q else 0 (valid causal region of a
            # diagonal tile in S^T = [k, q] layout).
            ut1 = singles.tile([P, P], MD)
            make_upper_triangular(nc, ut1[:], val=1.0, diag=True)

            bq_sb = singles.tile([P, NM], F32)
            nc.sync.dma_start(bq_sb[:], bq_d.rearrange("(m p) -> p m", p=P))
            bk_sb = singles.tile([P, NM], F32)
            nc.sync.dma_start(bk_sb[:], bk_d.rearrange("(m p) -> p m", p=P))
            # bv broadcast to all partitions via step-0 partition DMA read.
            bv_sb = singles.tile([P, CV], F32)
            nc.sync.dma_start(
                bv_sb[:], bass.AP(tensor=bv_d, offset=0, ap=[[0, P], [1, CV]])
            )

            wqk_pool = ctx.enter_context(tc.tile_pool(name="wqk", bufs=1))
            wq_sb = wqk_pool.tile([P, ND, C], MD)
            wk_sb = wqk_pool.tile([P, ND, C], MD)

            # PSUM: two score slots (2 banks each) + 2 O^T slots (2 banks
            # each) = 8 banks. Projection psums tag-share the score slots.
            ps_s0 = ctx.enter_context(tc.tile_pool(name="ps_s0", bufs=1, space="PSUM"))
            ps_s1 = ctx.enter_context(tc.tile_pool(name="ps_s1", bufs=1, space="PSUM"))
            ps_o = ctx.enter_context(tc.tile_pool(name="ps_o", bufs=2, space="PSUM"))

            def s_pool(i):
                return ps_s0 if i % 2 == 0 else ps_s1

            # Mid-kernel-released pools on the right-side stack: wv first,
            # xT last.
            xT_ctx = ExitStack()
            xT_pool = xT_ctx.enter_context(tc.tile_pool(name="xT", bufs=1, side="right"))
            wv_ctx = ExitStack()
            wv_pool = wv_ctx.enter_context(tc.tile_pool(name="wv", bufs=1, side="right"))

            xT = xT_pool.tile([P, ND, S], MD)
            wv_sb = wv_pool.tile([P, ND, CV], MD)
            nc.sync.dma_start(wv_sb[:], wv_d.rearrange("(o p) n -> p o n", p=P))
            # x^T arrives in token-quarters so V tiles can start early.
            xT_src = xT_d.rearrange("(o p) n -> p o n", p=P)
            TQ = S // 4
            for i in range(4):
                nc.sync.dma_start(
                    xT[:, :, i * TQ:(i + 1) * TQ], xT_src[:, :, i * TQ:(i + 1) * TQ]
                )

            v_pool = ctx.enter_context(tc.tile_pool(name="v", bufs=1))
            v_sb = v_pool.tile([P, NT, HN, VW], MD)
            v_pieces = [(lo, min(512, CV - lo)) for lo in range(0, CV, 512)]

            slot = [0]   # shared psum score-slot parity counter

            def emit_v_tile(t):
                psv = s_pool(slot[0]).tile([P, CV], F32, tag=f"s{slot[0] % 2}", name="psv")
                slot[0] += 1
                for d in range(ND):
                    for lo, w in v_pieces:
                        nc.tensor.matmul(
                            psv[:, lo:lo + w], xT[:, d, t * P:(t + 1) * P],
                            wv_sb[:, d, lo:lo + w],
                            start=(d == 0), stop=(d == ND - 1),
                        )
                nc.vector.tensor_add(v_sb[:, t], psv[:], bv_sb[:])

            # V projection for the first NT//2 token tiles runs up front;
            # the rest interleaves into head-pair 0's first window.
            for t in range(NT // 2):
                emit_v_tile(t)

            # Q/K weights after x/wv so the V pipeline starts sooner.
            nc.sync.dma_start(wq_sb[:], wq_d.rearrange("(o p) n -> p o n", p=P))
            nc.sync.dma_start(wk_sb[:], wk_d.rearrange("(o p) n -> p o n", p=P))

            wo_pool = ctx.enter_context(tc.tile_pool(name="wo", bufs=1))
            wo_sb = wo_pool.tile([P, NM, D], MD)
            nc.sync.dma_start(wo_sb[:], wo_d.rearrange("(f p) n -> p f n", p=P))

            # normalized O^T tiles until phase D; rows 0-63 = odd head (rr=1,
            # direct write), rows 64-127 = even head (rr=0, via DMA). wo rows
            # are swapped host-side to match.
            stg_pool = ctx.enter_context(tc.tile_pool(name="stg", bufs=1))
            stg_all = stg_pool.tile([P, NQC, NM, QW], MD)
            qkT_pool = ctx.enter_context(tc.tile_pool(name="qkT", bufs=2))
            pT_pool = ctx.enter_context(tc.tile_pool(name="pT", bufs=4))
            norm_pool = ctx.enter_context(tc.tile_pool(name="norm", bufs=2))
            sums_dram = ctx.enter_context(tc.tile_pool(name="sumsd", bufs=4, space="DRAM"))
            ostg_pool = ctx.enter_context(tc.tile_pool(name="ostg", bufs=3))

            def c1_emitters(hp):
                """Q^T/K^T projection for head-pair hp: per-head [64, S]
                tiles plus 8 emitter closures (one per psum accumulation
                group) for interleaved emission."""
                qT0 = qkT_pool.tile([HD, S], MD, tag="qT0", name="qT0")
                qT1 = qkT_pool.tile([HD, S], MD, tag="qT1", name="qT1")
                kT0 = qkT_pool.tile([HD, S], MD, tag="kT0", name="kT0")
                kT1 = qkT_pool.tile([HD, S], MD, tag="kT1", name="kT1")
                qTs = [qT0[:], qT1[:]]
                kTs = [kT0[:], kT1[:]]
                emitters = []
                for n in range(S // W5):
                    for w_sb, b_sb, dsts in ((wq_sb, bq_sb, qTs), (wk_sb, bk_sb, kTs)):
                        def emit(n=n, w_sb=w_sb, b_sb=b_sb, dsts=dsts):
                            sl = slice(n * W5, (n + 1) * W5)
                            psq = s_pool(slot[0]).tile(
                                [P, W5], F32, tag=f"s{slot[0] % 2}", name="psq"
                            )
                            slot[0] += 1
                            for d in range(ND):
                                nc.tensor.matmul(
                                    psq[:], w_sb[:, d, hp * P:(hp + 1) * P],
                                    xT[:, d, sl],
                                    start=(d == 0), stop=(d == ND - 1),
                                )
                            nc.vector.tensor_scalar_add(
                                dsts[0][:, sl], psq[0:HD, :], b_sb[0:HD, hp:hp + 1])
                            nc.vector.tensor_scalar_add(
                                dsts[1][:, sl], psq[HD:P, :], b_sb[HD:P, hp:hp + 1])
                        emitters.append(emit)
                return qTs, kTs, emitters

            d_done = set()
            d_fill = []   # phase-D pso closures, pumped as PE filler

            def queue_d_chunk(half, qc):
                """Partial output projection for q-chunk qc over head pairs
                f in {2*half, 2*half+1}: 16 independent pso closures."""
                if (half, qc) in d_done:
                    return
                d_done.add((half, qc))
                dst_d = out01_d if half == 0 else out23_d
                fs = (2 * half, 2 * half + 1)
                for mm in range(QW // P):
                    m = qc * (QW // P) + mm
                    off = mm * P
                    for n in range(D // WD):
                        def go(m=m, off=off, n=n, qc=qc, fs=fs, dst_d=dst_d):
                            pso = s_pool(slot[0]).tile(
                                [P, WD], F32, tag=f"s{slot[0] % 2}", name="pso"
                            )
                            slot[0] += 1
                            for j, f in enumerate(fs):
                                nc.tensor.matmul(
                                    pso[:], stg_all[:, qc, f, off:off + P],
                                    wo_sb[:, f, n * WD:(n + 1) * WD],
                                    start=(j == 0), stop=(j == len(fs) - 1),
                                )
                            ost = ostg_pool.tile([P, WD], F32, tag="ostg", name="ost")
                            nc.vector.tensor_copy(ost[:], pso[:])
                            nc.sync.dma_start(
                                dst_d[m * P:(m + 1) * P, n * WD:(n + 1) * WD], ost[:]
                            )
                        d_fill.append(go)

            def norm_chunk(hp, rr, qc, o_acc):
                """Normalize finished O^T chunk: rows 0..HD-1 scaled by
                1/sums (row HD), written into stg. O^T and the sums row are
                evacuated to SBUF right away (frees the PSUM slot); the
                multiply runs on the otherwise-idle GpSimd engine. rr=1
                writes stg rows 0-63 directly; rr=0 goes via tmpn + a
                partition-shift DMA."""
                oc = norm_pool.tile([HD, QW], F32, tag="oc", name="oc")
                nc.vector.tensor_copy(oc[:], o_acc[0:HD, :])
                srow = norm_pool.tile([1, QW], F32, tag="srow", name="srow")
                nc.vector.tensor_copy(srow[:], o_acc[HD:VW, :])
                rd1 = sums_dram.tile([1, QW], F32, tag="rd1", name="rd1")
                rd1_ap = rd1[:]
                nc.sync.dma_start(rd1_ap, srow[:])
                sq = norm_pool.tile([P, SQF], F32, tag="sq", name="sq")
                nc.sync.dma_start(
                    sq[:],
                    bass.AP(tensor=rd1_ap.tensor, offset=rd1_ap.offset,
                            ap=[[SQF, P], [1, SQF]]),
                )
                nc.vector.reciprocal(sq[:], sq[:])
                rd2 = sums_dram.tile([1, QW], F32, tag="rd2", name="rd2")
                rd2_ap = rd2[:]
                nc.sync.dma_start(
                    bass.AP(tensor=rd2_ap.tensor, offset=rd2_ap.offset,
                            ap=[[SQF, P], [1, SQF]]),
                    sq[:],
                )
                bc = norm_pool.tile([HD, QW], F32, tag="bc", name="bc")
                nc.sync.dma_start(
                    bc[:],
                    bass.AP(tensor=rd2_ap.tensor, offset=rd2_ap.offset,
                            ap=[[0, HD], [1, QW]]),
                )
                if rr == 1:
                    nc.gpsimd.tensor_mul(
                        stg_all[0:HD, qc, hp, :], oc[:], bc[:]
                    )
                else:
                    tmpn = norm_pool.tile([HD, QW], MD, tag="tmpn", name="tmpn")
                    nc.gpsimd.tensor_mul(tmpn[:], oc[:], bc[:])
                    nc.sync.dma_start(stg_all[HD:P, qc, hp, :], tmpn[:])

            # ---- attention over head pairs -------------------------------
            cur = c1_emitters(0)
            for em in cur[2]:
                em()

            for hp in range(NM):
                qTs, kTs = cur[0], cur[1]
                if hp + 1 < NM:
                    nxt = c1_emitters(hp + 1)
                else:
                    nxt = None
                    xT_ctx.close()

                pend_pv = [None]  # delayed-by-one-step PV emitter

                def emit_scores(rr, ch, ki):
                    W0 = ch * SW
                    qlo = max(W0, ki * P)
                    rel = qlo - W0
                    s_ps = s_pool(slot[0]).tile(
                        [P, SW], F32, tag=f"s{slot[0] % 2}", name="s_ps"
                    )
                    slot[0] += 1
                    plo = rel
                    while plo < SW:
                        pw = min(512 - plo % 512, SW - plo)
                        nc.tensor.matmul(
                            s_ps[:, plo:plo + pw],
                            kTs[rr][:, ki * P:(ki + 1) * P],
                            qTs[rr][:, W0 + plo:W0 + plo + pw],
                            start=True, stop=True,
                        )
                        plo += pw
                    pT = pT_pool.tile([P, SW], MD, tag="pT", name="pT")
                    nc.scalar.activation(
                        pT[:, rel:SW], s_ps[:, rel:SW],
                        mybir.ActivationFunctionType.Exp, scale=SCALE,
                    )
                    if ki * P >= W0:
                        # diagonal tile: zero the strictly-lower part (on the
                        # otherwise-idle GpSimd engine, off the DVE queue)
                        nc.gpsimd.tensor_mul(
                            pT[:, rel:rel + P], pT[:, rel:rel + P], ut1[:]
                        )
                    return pT, rel

                def make_pv(rr, ch, ki, pT, rel, o_ps):
                    def emit_pv():
                        W0 = ch * SW
                        h = 2 * hp + rr
                        plo = rel
                        done = False
                        while plo < SW:
                            pw = min(512 - plo % 512, SW - plo)
                            lastki = (ch + 1) * (SW // P) - 1
                            if o_ps[rr][ch] is None:
                                o_ps[rr][ch] = ps_o.tile(
                                    [VW, QW], F32, tag="o", name="o_ps"
                                )
                            nc.tensor.matmul(
                                o_ps[rr][ch][:, plo:plo + pw],
                                v_sb[:, ki, h, :],
                                pT[:, plo:plo + pw],
                                start=(ki == 0), stop=(ki == lastki),
                            )
                            plo += pw
                            done = ki == lastki
                        if done:
                            norm_chunk(hp, rr, ch, o_ps[rr][ch])
                            o_ps[rr][ch] = None
                            if rr == 1 and hp == 1:
                                # out01 chunk ready (f0 from hp0, f1 just now)
                                if ch == 0:
                                    queue_d_chunk(0, 0)
                                # qc1 of out01 is deferred to hp3-ch0 (below)
                            if rr == 1 and hp == NM - 1:
                                queue_d_chunk(1, ch)
                    return emit_pv

                # fillers per window: V tiles 8-15 go into (hp0, ch0); the
                # next head-pair's QK projection spreads over both windows;
                # out01's deferred qc1 tops up hp2-ch1 and hp3-ch0.
                fill = {0: [], 1: []}
                if hp == 0:
                    fill[0] = [lambda t=t: emit_v_tile(t) for t in range(NT // 2, NT)]
                    if nxt is not None:
                        fill[1] = list(nxt[2])
                elif nxt is not None:
                    fill[0] = list(nxt[2][:3])
                    fill[1] = list(nxt[2][3:])
                if hp == 2:
                    fill[1] = fill[1] + [lambda: queue_d_chunk(0, 1)]

                o_ps = [[None] * NCH for _ in range(2)]
                for ch in range(NCH):
                    nki = ((ch + 1) * SW) // P
                    steps = [(rr, ki) for ki in range(nki) for rr in (0, 1)]
                    fillers = fill[ch]
                    emitted = 0
                    for i, (rr, ki) in enumerate(steps):
                        pT, rel = emit_scores(rr, ch, ki)
                        if pend_pv[0] is not None:
                            pend_pv[0]()
                        pend_pv[0] = make_pv(rr, ch, ki, pT, rel, o_ps)
                        want = ((i + 1) * len(fillers)) // len(steps)
                        while emitted < want:
                            fillers[emitted]()
                            emitted += 1
                        if d_fill:
                            d_fill.pop(0)()
                # flush the delayed PV at the end of the head-pair
                if pend_pv[0] is not None:
                    pend_pv[0]()
                    pend_pv[0] = None
                if hp == 0:
                    wv_ctx.close()
                cur = nxt

            # ---- phase D remainder ---------------------------------------
            for half in range(2):
                for qc in range(NQC):
                    queue_d_chunk(half, qc)
            while d_fill:
                d_fill.pop(0)()

    nc.compile()
    return nc


_NC_CACHE = {}


def _get_nc(S, D, HN, HD):
    key = (S, D, HN, HD)
    if key not in _NC_CACHE:
        _NC_CACHE[key] = build_nc(S, D, HN, HD)
    return _NC_CACHE[key]


def augment_v(wv_local, bv_local, HN, HD):
    """Append per head a zero weight column with bias 1.0 (softmax-sum col)."""
    D = wv_local.shape[0]
    wv_a = np.zeros((D, HN, HD + 1), dtype=np.float32)
    wv_a[:, :, :HD] = wv_local.reshape(D, HN, HD)
    bv_a = np.ones((HN, HD + 1), dtype=np.float32)
    bv_a[:, :HD] = bv_local.reshape(HN, HD)
    return np.ascontiguousarray(wv_a.reshape(D, -1)), np.ascontiguousarray(bv_a.reshape(-1))


def kernel(**inputs):
    out, _ = run_with_results(inputs)
    return out


def run_with_results(inputs, **spmd_kwargs):
    from concourse.bass_utils import run_bass_kernel_spmd
    import ml_dtypes

    bf16 = ml_dtypes.bfloat16

    x = np.asarray(inputs["x"], dtype=np.float32)
    wq = np.asarray(inputs["wq"], dtype=np.float32)
    bq = np.asarray(inputs["bq"], dtype=np.float32)
    wk = np.asarray(inputs["wk"], dtype=np.float32)
    bk = np.asarray(inputs["bk"], dtype=np.float32)
    wv = np.asarray(inputs["wv"], dtype=np.float32)
    bv = np.asarray(inputs["bv"], dtype=np.float32)
    wo = np.asarray(inputs["wo"], dtype=np.float32)
    bo = np.asarray(inputs["bo"], dtype=np.float32)

    B, S, D = x.shape
    H = 16
    HD = D // H
    G = 2                  # head groups
    HN = H // G            # heads per core
    C = HN * HD
    n_cores = B * G

    nc = _get_nc(S, D, HN, HD)

    in_maps = []
    for c in range(n_cores):
        b, g = c // G, c % G
        sl = slice(g * C, (g + 1) * C)
        # stg packs the odd head in rows 0-63 and the even head in rows
        # 64-127 of each 128-row block; swap wo's rows to match.
        wo_loc = wo[sl, :].reshape(HN // 2, 2, HD, D)[:, ::-1]
        wo_loc = np.ascontiguousarray(wo_loc.reshape(C, D))
        wv_a, bv_a = augment_v(wv[:, sl], bv[sl], HN, HD)
        in_maps.append({
            "xT": np.ascontiguousarray(x[b].T).astype(bf16),
            "wq": np.ascontiguousarray(wq[:, sl]).astype(bf16),
            "wk": np.ascontiguousarray(wk[:, sl]).astype(bf16),
            "wv": wv_a.astype(bf16),
            "wo": wo_loc.astype(bf16),
            "bq": np.ascontiguousarray(bq[sl]),
            "bk": np.ascontiguousarray(bk[sl]),
            "bv": bv_a,
        })

    res = run_bass_kernel_spmd(nc, in_maps, core_ids=list(range(n_cores)), **spmd_kwargs)
    outs = [m["out01"] + m["out23"] for m in res.results]
    out = np.stack([sum(outs[b * G + g] for g in range(G)) for b in range(B)])
    out = out + bo[None, None, :]
    return out.astype(np.float32), res


# revision 24
# speedup vs baseline: 1.4084x; 1.4084x over previous
"""Trainium2 Bass kernel for nn_MultiHeadAttention (B=4, S=2048, D=1024, H=16).

Sharding: 8 cores = 4 batches x 2 head-groups. Core c handles batch c//2 and
heads (c%2)*8 .. (c%2)*8+7. Each core computes Q/K/V projections for its 512
columns, causal attention for its 8 heads, and two partial output
projections (row-parallel over wo, split f={0,1} / f={2,3}). Host sums the
partials per batch and adds bo.

Per-core layout: x^T [D, S] shipped pre-transposed (bf16). Q^T/K^T per
head-pair packed [128, S] float32r (head h=2hp+rr in rows 64*rr..64*rr+63;
scores contract over head_dim with base-partition-64 APs for the odd head —
the f32r path supports this). V natural [S, 65] per head with a trailing
ones column so P^T@V also produces softmax row-sums. S^T = K^T.T @ Q^T per
128-k-tile into [128, 1024] PSUM windows, exp on ACT, O^T accumulates in
[65, 1024] PSUM chunks (row 64 = sums).

Scheduling: the two heads of a pair interleave step-by-step (rr alternates
per ki) and each step's PV matmul is emitted one step late so the PE never
waits on the exp it just issued. Scores run in f32r (2x PE cycles) which
keeps the tensor engine saturated during ACT-paced stretches — the PE HAM
clock gate re-throttles to 1.2 GHz whenever PE duty drops, which costs more
than the extra f32r cycles. All deferrable GEMM work (V tiles 8-15, the
next head-pair's Q/K projection, finished out01/out23 chunks) is spread
evenly into the attention windows as PE filler. Normalization evacuates
O^T + sums to SBUF, reshapes sums via small DRAM round-trips for a cheap
[128, 8] reciprocal, and multiplies on the otherwise-idle GpSimd engine.
The odd (rr=1) head lands in stg rows 0-63 by a direct write (wo rows are
swapped host-side to match) since it finishes last.
"""

import numpy as np

import concourse.bass as bass
import concourse.mybir as mybir
import concourse.tile as tile
from concourse import bacc
from concourse.masks import make_upper_triangular

F32 = mybir.dt.float32
F32R = mybir.dt.float32r
BF16 = mybir.dt.bfloat16
P = 128


def build_nc(S=2048, D=1024, HN=8, HD=64):
    """Per-core Bass module. HN = local heads, C = HN*HD local cols."""
    MD = BF16
    C = HN * HD
    NT = S // P        # token tiles
    ND = D // P        # d tiles (contraction for projections)
    NM = C // P        # head-pairs (2 heads of HD=64 per 128-partition tile)
    QW = 1024          # O^T psum chunk width (two PSUM banks; chunk == window)
    SW = 1024          # scores window width (two PSUM banks)
    W5 = 512           # projection N-chunk width
    WD = 512           # phase-D N-chunk width
    NCH = S // SW      # score windows per head
    NQC = S // QW      # O^T chunks per head
    SQF = QW // P      # free size of the [P, SQF] reciprocal reshape
    SCALE = 1.0 / float(np.sqrt(HD))
    VW = HD + 1        # V columns per head incl. trailing ones column
    CV = HN * VW       # augmented V cols

    nc = bacc.Bacc("TRN2", target_bir_lowering=False)

    xT_d = nc.dram_tensor("xT", [D, S], MD, kind="ExternalInput")
    wq_d = nc.dram_tensor("wq", [D, C], MD, kind="ExternalInput")
    wk_d = nc.dram_tensor("wk", [D, C], MD, kind="ExternalInput")
    wv_d = nc.dram_tensor("wv", [D, CV], MD, kind="ExternalInput")
    wo_d = nc.dram_tensor("wo", [C, D], MD, kind="ExternalInput")
    bq_d = nc.dram_tensor("bq", [C], F32, kind="ExternalInput")
    bk_d = nc.dram_tensor("bk", [C], F32, kind="ExternalInput")
    bv_d = nc.dram_tensor("bv", [CV], F32, kind="ExternalInput")
    # out01 = heads pairs 0-1 partial, out23 = head pairs 2-3; host adds.
    out01_d = nc.dram_tensor("out01", [S, D], F32, kind="ExternalOutput")
    out23_d = nc.dram_tensor("out23", [S, D], F32, kind="ExternalOutput")

    with tile.TileContext(nc) as tc:
        from contextlib import ExitStack

        with ExitStack() as ctx:
            singles = ctx.enter_context(tc.tile_pool(name="singles", bufs=1))
            # ut1[k, q] = 1.0 where k <= q else 0 (valid causal region of a
            # diagonal tile in S^T = [k, q] layout).
            ut1 = singles.tile([P, P], MD)
            make_upper_triangular(nc, ut1[:], val=1.0, diag=True)

            bq_sb = singles.tile([P, NM], F32)
            nc.sync.dma_start(bq_sb[:], bq_d.rearrange("(m p) -> p m", p=P))
            bk_sb = singles.tile([P, NM], F32)
            nc.sync.dma_start(bk_sb[:], bk_d.rearrange("(m p) -> p m", p=P))
            # bv broadcast to all partitions via step-0 partition DMA read.
            bv_sb = singles.tile([P, CV], F32)
            nc.sync.dma_start(
                bv_sb[:], bass.AP(tensor=bv_d, offset=0, ap=[[0, P], [1, CV]])
            )

            wqk_pool = ctx.enter_context(tc.tile_pool(name="wqk", bufs=1))
            wq_sb = wqk_pool.tile([P, ND, C], MD)
            wk_sb = wqk_pool.tile([P, ND, C], MD)

            # PSUM: three score slots (2 banks each) + 1 O^T slot (2 banks)
            # = 8 banks. Projection psums tag-share the score slots; the
            # 3-deep rotation means every allocation reuses a slot whose
            # previous reader (exp, two steps back) has already finished, so
            # neither scores nor interleaved filler matmuls stall the
            # in-order PE queue. The single O slot works because the two
            # heads of a pair run window-sequentially (one O accumulation
            # live at a time).
            ps_s = [
                ctx.enter_context(tc.tile_pool(name=f"ps_s{j}", bufs=1, space="PSUM"))
                for j in range(3)
            ]
            ps_o = ctx.enter_context(tc.tile_pool(name="ps_o", bufs=1, space="PSUM"))

            def s_pool(i):
                return ps_s[i % 3]

            # Mid-kernel-released pools on the right-side stack: wv first,
            # xT last.
            xT_ctx = ExitStack()
            xT_pool = xT_ctx.enter_context(tc.tile_pool(name="xT", bufs=1, side="right"))
            wv_ctx = ExitStack()
            wv_pool = wv_ctx.enter_context(tc.tile_pool(name="wv", bufs=1, side="right"))

            xT = xT_pool.tile([P, ND, S], MD)
            wv_sb = wv_pool.tile([P, ND, CV], MD)
            nc.sync.dma_start(wv_sb[:], wv_d.rearrange("(o p) n -> p o n", p=P))
            # x^T arrives in token-quarters so V tiles can start early.
            xT_src = xT_d.rearrange("(o p) n -> p o n", p=P)
            TQ = S // 4
            for i in range(4):
                nc.sync.dma_start(
                    xT[:, :, i * TQ:(i + 1) * TQ], xT_src[:, :, i * TQ:(i + 1) * TQ]
                )

            v_pool = ctx.enter_context(tc.tile_pool(name="v", bufs=1))
            v_sb = v_pool.tile([P, NT, HN, VW], MD)
            v_pieces = [(lo, min(512, CV - lo)) for lo in range(0, CV, 512)]

            slot = [0]   # shared psum score-slot parity counter

            def emit_v_tile(t):
                psv = s_pool(slot[0]).tile([P, CV], F32, tag=f"s{slot[0] % 3}", name="psv")
                slot[0] += 1
                for d in range(ND):
                    for lo, w in v_pieces:
                        nc.tensor.matmul(
                            psv[:, lo:lo + w], xT[:, d, t * P:(t + 1) * P],
                            wv_sb[:, d, lo:lo + w],
                            start=(d == 0), stop=(d == ND - 1),
                        )
                nc.vector.tensor_add(v_sb[:, t], psv[:], bv_sb[:])

            # V projection for the first NT//2 token tiles runs up front;
            # the rest interleaves into head-pair 0's first window.
            for t in range(NT // 2):
                emit_v_tile(t)

            # Q/K weights after x/wv so the V pipeline starts sooner.
            nc.sync.dma_start(wq_sb[:], wq_d.rearrange("(o p) n -> p o n", p=P))
            nc.sync.dma_start(wk_sb[:], wk_d.rearrange("(o p) n -> p o n", p=P))

            wo_pool = ctx.enter_context(tc.tile_pool(name="wo", bufs=1))
            wo_sb = wo_pool.tile([P, NM, D], MD)
            nc.sync.dma_start(wo_sb[:], wo_d.rearrange("(f p) n -> p f n", p=P))

            # normalized O^T tiles until phase D; rows 0-63 = odd head (rr=1,
            # direct write), rows 64-127 = even head (rr=0, via DMA). wo rows
            # are swapped host-side to match.
            stg_pool = ctx.enter_context(tc.tile_pool(name="stg", bufs=1))
            stg_all = stg_pool.tile([P, NQC, NM, QW], MD)
            qkT_pool = ctx.enter_context(tc.tile_pool(name="qkT", bufs=2))
            pT_pool = ctx.enter_context(tc.tile_pool(name="pT", bufs=4))
            norm_pool = ctx.enter_context(tc.tile_pool(name="norm", bufs=2))
            sums_dram = ctx.enter_context(tc.tile_pool(name="sumsd", bufs=4, space="DRAM"))
            ostg_pool = ctx.enter_context(tc.tile_pool(name="ostg", bufs=3))

            def c1_emitters(hp):
                """Q^T/K^T projection for head-pair hp: per-head [64, S]
                tiles plus 8 emitter closures (one per psum accumulation
                group) for interleaved emission."""
                qT0 = qkT_pool.tile([HD, S], MD, tag="qT0", name="qT0")
                qT1 = qkT_pool.tile([HD, S], MD, tag="qT1", name="qT1")
                kT0 = qkT_pool.tile([HD, S], MD, tag="kT0", name="kT0")
                kT1 = qkT_pool.tile([HD, S], MD, tag="kT1", name="kT1")
                qTs = [qT0[:], qT1[:]]
                kTs = [kT0[:], kT1[:]]
                emitters = []
                for n in range(S // W5):
                    for w_sb, b_sb, dsts in ((wq_sb, bq_sb, qTs), (wk_sb, bk_sb, kTs)):
                        def emit(n=n, w_sb=w_sb, b_sb=b_sb, dsts=dsts):
                            sl = slice(n * W5, (n + 1) * W5)
                            psq = s_pool(slot[0]).tile(
                                [P, W5], F32, tag=f"s{slot[0] % 3}", name="psq"
                            )
                            slot[0] += 1
                            for d in range(ND):
                                nc.tensor.matmul(
                                    psq[:], w_sb[:, d, hp * P:(hp + 1) * P],
                                    xT[:, d, sl],
                                    start=(d == 0), stop=(d == ND - 1),
                                )
                            nc.vector.tensor_scalar_add(
                                dsts[0][:, sl], psq[0:HD, :], b_sb[0:HD, hp:hp + 1])
                            nc.vector.tensor_scalar_add(
                                dsts[1][:, sl], psq[HD:P, :], b_sb[HD:P, hp:hp + 1])
                        emitters.append(emit)
                return qTs, kTs, emitters

            d_done = set()
            d_fill = []   # phase-D pso closures, pumped as PE filler

            def queue_d_chunk(half, qc):
                """Partial output projection for q-chunk qc over head pairs
                f in {2*half, 2*half+1}: 16 independent pso closures."""
                if (half, qc) in d_done:
                    return
                d_done.add((half, qc))
                dst_d = out01_d if half == 0 else out23_d
                fs = (2 * half, 2 * half + 1)
                for mm in range(QW // P):
                    m = qc * (QW // P) + mm
                    off = mm * P
                    for n in range(D // WD):
                        def go(m=m, off=off, n=n, qc=qc, fs=fs, dst_d=dst_d):
                            pso = s_pool(slot[0]).tile(
                                [P, WD], F32, tag=f"s{slot[0] % 3}", name="pso"
                            )
                            slot[0] += 1
                            for j, f in enumerate(fs):
                                nc.tensor.matmul(
                                    pso[:], stg_all[:, qc, f, off:off + P],
                                    wo_sb[:, f, n * WD:(n + 1) * WD],
                                    start=(j == 0), stop=(j == len(fs) - 1),
                                )
                            ost = ostg_pool.tile([P, WD], F32, tag="ostg", name="ost")
                            nc.vector.tensor_copy(ost[:], pso[:])
                            nc.sync.dma_start(
                                dst_d[m * P:(m + 1) * P, n * WD:(n + 1) * WD], ost[:]
                            )
                        d_fill.append(go)

            def norm_chunk(hp, rr, qc, o_acc):
                """Normalize finished O^T chunk: rows 0..HD-1 scaled by
                1/sums (row HD), written into stg. O^T and the sums row are
                evacuated to SBUF right away (frees the PSUM slot); the
                multiply runs on the otherwise-idle GpSimd engine. rr=1
                writes stg rows 0-63 directly; rr=0 goes via tmpn + a
                partition-shift DMA."""
                oc = norm_pool.tile([HD, QW], F32, tag="oc", name="oc")
                nc.vector.tensor_copy(oc[:], o_acc[0:HD, :])
                srow = norm_pool.tile([1, QW], F32, tag="srow", name="srow")
                nc.vector.tensor_copy(srow[:], o_acc[HD:VW, :])
                rd1 = sums_dram.tile([1, QW], F32, tag="rd1", name="rd1")
                rd1_ap = rd1[:]
                nc.sync.dma_start(rd1_ap, srow[:])
                sq = norm_pool.tile([P, SQF], F32, tag="sq", name="sq")
                nc.sync.dma_start(
                    sq[:],
                    bass.AP(tensor=rd1_ap.tensor, offset=rd1_ap.offset,
                            ap=[[SQF, P], [1, SQF]]),
                )
                nc.vector.reciprocal(sq[:], sq[:])
                rd2 = sums_dram.tile([1, QW], F32, tag="rd2", name="rd2")
                rd2_ap = rd2[:]
                nc.sync.dma_start(
                    bass.AP(tensor=rd2_ap.tensor, offset=rd2_ap.offset,
                            ap=[[SQF, P], [1, SQF]]),
                    sq[:],
                )
                bc = norm_pool.tile([HD, QW], F32, tag="bc", name="bc")
                nc.sync.dma_start(
                    bc[:],
                    bass.AP(tensor=rd2_ap.tensor, offset=rd2_ap.offset,
                            ap=[[0, HD], [1, QW]]),
                )
                if rr == 1:
                    nc.gpsimd.tensor_mul(
                        stg_all[0:HD, qc, hp, :], oc[:], bc[:]
                    )
                else:
                    tmpn = norm_pool.tile([HD, QW], MD, tag="tmpn", name="tmpn")
                    nc.gpsimd.tensor_mul(tmpn[:], oc[:], bc[:])
                    nc.sync.dma_start(stg_all[HD:P, qc, hp, :], tmpn[:])

            # ---- attention over head pairs -------------------------------
            cur = c1_emitters(0)
            for em in cur[2]:
                em()

            for hp in range(NM):
                qTs, kTs = cur[0], cur[1]
                if hp + 1 < NM:
                    nxt = c1_emitters(hp + 1)
                else:
                    nxt = None
                    xT_ctx.close()

                pend_pv = [None]  # delayed-by-one-step PV emitter

                def emit_scores(rr, ch, ki):
                    W0 = ch * SW
                    qlo = max(W0, ki * P)
                    rel = qlo - W0
                    s_ps = s_pool(slot[0]).tile(
                        [P, SW], F32, tag=f"s{slot[0] % 3}", name="s_ps"
                    )
                    slot[0] += 1
                    plo = rel
                    while plo < SW:
                        pw = min(512 - plo % 512, SW - plo)
                        nc.tensor.matmul(
                            s_ps[:, plo:plo + pw],
                            kTs[rr][:, ki * P:(ki + 1) * P],
                            qTs[rr][:, W0 + plo:W0 + plo + pw],
                            start=True, stop=True,
                        )
                        plo += pw
                    pT = pT_pool.tile([P, SW], MD, tag="pT", name="pT")
                    nc.scalar.activation(
                        pT[:, rel:SW], s_ps[:, rel:SW],
                        mybir.ActivationFunctionType.Exp, scale=SCALE,
                    )
                    if ki * P >= W0:
                        # diagonal tile: zero the strictly-lower part (on the
                        # otherwise-idle GpSimd engine, off the DVE queue)
                        nc.gpsimd.tensor_mul(
                            pT[:, rel:rel + P], pT[:, rel:rel + P], ut1[:]
                        )
                    return pT, rel

                def make_pv(rr, ch, ki, pT, rel, o_ps):
                    def emit_pv():
                        W0 = ch * SW
                        h = 2 * hp + rr
                        plo = rel
                        done = False
                        while plo < SW:
                            pw = min(512 - plo % 512, SW - plo)
                            lastki = (ch + 1) * (SW // P) - 1
                            if o_ps[rr][ch] is None:
                                o_ps[rr][ch] = ps_o.tile(
                                    [VW, QW], F32, tag="o", name="o_ps"
                                )
                            nc.tensor.matmul(
                                o_ps[rr][ch][:, plo:plo + pw],
                                v_sb[:, ki, h, :],
                                pT[:, plo:plo + pw],
                                start=(ki == 0), stop=(ki == lastki),
                            )
                            plo += pw
                            done = ki == lastki
                        if done:
                            norm_chunk(hp, rr, ch, o_ps[rr][ch])
                            o_ps[rr][ch] = None
                            if rr == 1 and hp == 1:
                                # out01 chunk ready (f0 from hp0, f1 just now)
                                if ch == 0:
                                    queue_d_chunk(0, 0)
                                # qc1 of out01 is deferred to hp3-ch0 (below)
                            if rr == 1 and hp == NM - 1:
                                queue_d_chunk(1, ch)
                    return emit_pv

                # fillers per window: V tiles 8-15 go into (hp0, ch0); the
                # next head-pair's QK projection spreads over both windows;
                # out01's deferred qc1 tops up hp2-ch1 and hp3-ch0.
                fill = {0: [], 1: []}
                if hp == 0:
                    fill[0] = [lambda t=t: emit_v_tile(t) for t in range(NT // 2, NT)]
                    if nxt is not None:
                        fill[1] = list(nxt[2])
                elif nxt is not None:
                    fill[0] = list(nxt[2][:3])
                    fill[1] = list(nxt[2][3:])
                if hp == 2:
                    fill[1] = fill[1] + [lambda: queue_d_chunk(0, 1)]

                o_ps = [[None] * NCH for _ in range(2)]
                for ch in range(NCH):
                    nki = ((ch + 1) * SW) // P
                    # window-sequential heads: all of rr=0's k-tiles, then
                    # rr=1's — keeps a single O^T accumulation live at a time
                    steps = [(rr, ki) for rr in (0, 1) for ki in range(nki)]
                    fillers = fill[ch]
                    emitted = 0
                    for i, (rr, ki) in enumerate(steps):
                        pT, rel = emit_scores(rr, ch, ki)
                        if pend_pv[0] is not None:
                            pend_pv[0]()
                        pend_pv[0] = make_pv(rr, ch, ki, pT, rel, o_ps)
                        want = ((i + 1) * len(fillers)) // len(steps)
                        while emitted < want:
                            fillers[emitted]()
                            emitted += 1
                        if d_fill and i % 2 == 0:
                            d_fill.pop(0)()
                # flush the delayed PV at the end of the head-pair
                if pend_pv[0] is not None:
                    pend_pv[0]()
                    pend_pv[0] = None
                if hp == 0:
                    wv_ctx.close()
                cur = nxt

            # ---- phase D remainder ---------------------------------------
            for half in range(2):
                for qc in range(NQC):
                    queue_d_chunk(half, qc)
            while d_fill:
                d_fill.pop(0)()

    nc.compile()
    return nc


_NC_CACHE = {}


def _get_nc(S, D, HN, HD):
    key = (S, D, HN, HD)
    if key not in _NC_CACHE:
        _NC_CACHE[key] = build_nc(S, D, HN, HD)
    return _NC_CACHE[key]


def augment_v(wv_local, bv_local, HN, HD):
    """Append per head a zero weight column with bias 1.0 (softmax-sum col)."""
    D = wv_local.shape[0]
    wv_a = np.zeros((D, HN, HD + 1), dtype=np.float32)
    wv_a[:, :, :HD] = wv_local.reshape(D, HN, HD)
    bv_a = np.ones((HN, HD + 1), dtype=np.float32)
    bv_a[:, :HD] = bv_local.reshape(HN, HD)
    return np.ascontiguousarray(wv_a.reshape(D, -1)), np.ascontiguousarray(bv_a.reshape(-1))


def kernel(**inputs):
    out, _ = run_with_results(inputs)
    return out


def run_with_results(inputs, **spmd_kwargs):
    from concourse.bass_utils import run_bass_kernel_spmd
    import ml_dtypes

    bf16 = ml_dtypes.bfloat16

    x = np.asarray(inputs["x"], dtype=np.float32)
    wq = np.asarray(inputs["wq"], dtype=np.float32)
    bq = np.asarray(inputs["bq"], dtype=np.float32)
    wk = np.asarray(inputs["wk"], dtype=np.float32)
    bk = np.asarray(inputs["bk"], dtype=np.float32)
    wv = np.asarray(inputs["wv"], dtype=np.float32)
    bv = np.asarray(inputs["bv"], dtype=np.float32)
    wo = np.asarray(inputs["wo"], dtype=np.float32)
    bo = np.asarray(inputs["bo"], dtype=np.float32)

    B, S, D = x.shape
    H = 16
    HD = D // H
    G = 2                  # head groups
    HN = H // G            # heads per core
    C = HN * HD
    n_cores = B * G

    nc = _get_nc(S, D, HN, HD)

    in_maps = []
    for c in range(n_cores):
        b, g = c // G, c % G
        sl = slice(g * C, (g + 1) * C)
        # stg packs the odd head in rows 0-63 and the even head in rows
        # 64-127 of each 128-row block; swap wo's rows to match.
        wo_loc = wo[sl, :].reshape(HN // 2, 2, HD, D)[:, ::-1]
        wo_loc = np.ascontiguousarray(wo_loc.reshape(C, D))
        wv_a, bv_a = augment_v(wv[:, sl], bv[sl], HN, HD)
        in_maps.append({
            "xT": np.ascontiguousarray(x[b].T).astype(bf16),
            "wq": np.ascontiguousarray(wq[:, sl]).astype(bf16),
            "wk": np.ascontiguousarray(wk[:, sl]).astype(bf16),
            "wv": wv_a.astype(bf16),
            "wo": wo_loc.astype(bf16),
            "bq": np.ascontiguousarray(bq[sl]),
            "bk": np.ascontiguousarray(bk[sl]),
            "bv": bv_a,
        })

    res = run_bass_kernel_spmd(nc, in_maps, core_ids=list(range(n_cores)), **spmd_kwargs)
    outs = [m["out01"] + m["out23"] for m in res.results]
    out = np.stack([sum(outs[b * G + g] for g in range(G)) for b in range(B)])
    out = out + bo[None, None, :]
    return out.astype(np.float32), res


# revision 31
# speedup vs baseline: 1.4095x; 1.0008x over previous
"""Trainium2 Bass kernel for nn_MultiHeadAttention (B=4, S=2048, D=1024, H=16).

Sharding: 8 cores = 4 batches x 2 head-groups. Core c handles batch c//2 and
heads (c%2)*8 .. (c%2)*8+7. Each core computes Q/K/V projections for its 512
columns, causal attention for its 8 heads, and two partial output
projections (row-parallel over wo, split f={0,1} / f={2,3}). Host sums the
partials per batch and adds bo.

Per-core layout: x^T [D, S] shipped pre-transposed (bf16). Q^T/K^T per
head-pair packed [128, S] float32r (head h=2hp+rr in rows 64*rr..64*rr+63;
scores contract over head_dim with base-partition-64 APs for the odd head —
the f32r path supports this). V natural [S, 65] per head with a trailing
ones column so P^T@V also produces softmax row-sums. S^T = K^T.T @ Q^T per
128-k-tile into [128, 1024] PSUM windows, exp on ACT, O^T accumulates in
[65, 1024] PSUM chunks (row 64 = sums).

Scheduling: the two heads of a pair interleave step-by-step (rr alternates
per ki) and each step's PV matmul is emitted one step late so the PE never
waits on the exp it just issued. Scores run in f32r (2x PE cycles) which
keeps the tensor engine saturated during ACT-paced stretches — the PE HAM
clock gate re-throttles to 1.2 GHz whenever PE duty drops, which costs more
than the extra f32r cycles. All deferrable GEMM work (V tiles 8-15, the
next head-pair's Q/K projection, finished out01/out23 chunks) is spread
evenly into the attention windows as PE filler. Normalization evacuates
O^T + sums to SBUF, reshapes sums via small DRAM round-trips for a cheap
[128, 8] reciprocal, and multiplies on the otherwise-idle GpSimd engine.
The odd (rr=1) head lands in stg rows 0-63 by a direct write (wo rows are
swapped host-side to match) since it finishes last.
"""

import numpy as np

import concourse.bass as bass
import concourse.mybir as mybir
import concourse.tile as tile
from concourse import bacc
from concourse.masks import make_upper_triangular

F32 = mybir.dt.float32
F32R = mybir.dt.float32r
BF16 = mybir.dt.bfloat16
P = 128


def build_nc(S=2048, D=1024, HN=8, HD=64):
    """Per-core Bass module. HN = local heads, C = HN*HD local cols."""
    MD = BF16
    C = HN * HD
    NT = S // P        # token tiles
    ND = D // P        # d tiles (contraction for projections)
    NM = C // P        # head-pairs (2 heads of HD=64 per 128-partition tile)
    QW = 1024          # O^T psum chunk width (two PSUM banks; chunk == window)
    SW = 1024          # scores window width (two PSUM banks)
    W5 = 512           # projection N-chunk width
    WD = 512           # phase-D N-chunk width
    NCH = S // SW      # score windows per head
    NQC = S // QW      # O^T chunks per head
    SQF = QW // P      # free size of the [P, SQF] reciprocal reshape
    SCALE = 1.0 / float(np.sqrt(HD))
    VW = HD + 1        # V columns per head incl. trailing ones column
    CV = HN * VW       # augmented V cols

    nc = bacc.Bacc("TRN2", target_bir_lowering=False)

    xT_d = nc.dram_tensor("xT", [D, S], MD, kind="ExternalInput")
    wq_d = nc.dram_tensor("wq", [D, C], MD, kind="ExternalInput")
    wk_d = nc.dram_tensor("wk", [D, C], MD, kind="ExternalInput")
    wv_d = nc.dram_tensor("wv", [D, CV], MD, kind="ExternalInput")
    wo_d = nc.dram_tensor("wo", [C, D], MD, kind="ExternalInput")
    bq_d = nc.dram_tensor("bq", [C], F32, kind="ExternalInput")
    bk_d = nc.dram_tensor("bk", [C], F32, kind="ExternalInput")
    bv_d = nc.dram_tensor("bv", [CV], F32, kind="ExternalInput")
    # out01 = heads pairs 0-1 partial, out23 = head pairs 2-3; host adds.
    out01_d = nc.dram_tensor("out01", [S, D], F32, kind="ExternalOutput")
    out23_d = nc.dram_tensor("out23", [S, D], F32, kind="ExternalOutput")

    with tile.TileContext(nc) as tc:
        from contextlib import ExitStack

        with ExitStack() as ctx:
            singles = ctx.enter_context(tc.tile_pool(name="singles", bufs=1))
            # ut1[k, q] = 1.0 where k <= q else 0 (valid causal region of a
            # diagonal tile in S^T = [k, q] layout).
            ut1 = singles.tile([P, P], MD)
            make_upper_triangular(nc, ut1[:], val=1.0, diag=True)

            bq_sb = singles.tile([P, NM], F32)
            nc.sync.dma_start(bq_sb[:], bq_d.rearrange("(m p) -> p m", p=P))
            bk_sb = singles.tile([P, NM], F32)
            nc.sync.dma_start(bk_sb[:], bk_d.rearrange("(m p) -> p m", p=P))
            # bv broadcast to all partitions via step-0 partition DMA read.
            bv_sb = singles.tile([P, CV], F32)
            nc.sync.dma_start(
                bv_sb[:], bass.AP(tensor=bv_d, offset=0, ap=[[0, P], [1, CV]])
            )

            wqk_pool = ctx.enter_context(tc.tile_pool(name="wqk", bufs=1))
            wq_sb = wqk_pool.tile([P, ND, C], MD)
            wk_sb = wqk_pool.tile([P, ND, C], MD)

            # PSUM: three score slots (2 banks each) + 1 O^T slot (2 banks)
            # = 8 banks. Projection psums tag-share the score slots; the
            # 3-deep rotation means every allocation reuses a slot whose
            # previous reader (exp, two steps back) has already finished, so
            # neither scores nor interleaved filler matmuls stall the
            # in-order PE queue. The single O slot works because the two
            # heads of a pair run window-sequentially (one O accumulation
            # live at a time).
            ps_s = [
                ctx.enter_context(tc.tile_pool(name=f"ps_s{j}", bufs=1, space="PSUM"))
                for j in range(3)
            ]
            ps_o = ctx.enter_context(tc.tile_pool(name="ps_o", bufs=1, space="PSUM"))

            def s_pool(i):
                return ps_s[i % 3]

            # Mid-kernel-released pools on the right-side stack: wv first,
            # xT last.
            xT_ctx = ExitStack()
            xT_pool = xT_ctx.enter_context(tc.tile_pool(name="xT", bufs=1, side="right"))
            wv_ctx = ExitStack()
            wv_pool = wv_ctx.enter_context(tc.tile_pool(name="wv", bufs=1, side="right"))

            xT = xT_pool.tile([P, ND, S], MD)
            wv_sb = wv_pool.tile([P, ND, CV], MD)
            nc.sync.dma_start(wv_sb[:], wv_d.rearrange("(o p) n -> p o n", p=P))
            # x^T arrives in token-quarters so V tiles can start early.
            xT_src = xT_d.rearrange("(o p) n -> p o n", p=P)
            TQ = S // 4
            for i in range(4):
                nc.sync.dma_start(
                    xT[:, :, i * TQ:(i + 1) * TQ], xT_src[:, :, i * TQ:(i + 1) * TQ]
                )

            v_pool = ctx.enter_context(tc.tile_pool(name="v", bufs=1))
            v_sb = v_pool.tile([P, NT, HN, VW], MD)
            v_pieces = [(lo, min(512, CV - lo)) for lo in range(0, CV, 512)]

            slot = [0]   # shared psum score-slot parity counter

            def emit_v_tile(t):
                psv = s_pool(slot[0]).tile([P, CV], F32, tag=f"s{slot[0] % 3}", name="psv")
                slot[0] += 1
                for d in range(ND):
                    for lo, w in v_pieces:
                        nc.tensor.matmul(
                            psv[:, lo:lo + w], xT[:, d, t * P:(t + 1) * P],
                            wv_sb[:, d, lo:lo + w],
                            start=(d == 0), stop=(d == ND - 1),
                        )
                nc.vector.tensor_add(v_sb[:, t], psv[:], bv_sb[:])

            # V projection for the first NT//2 token tiles runs up front;
            # the rest interleaves into head-pair 0's first window.
            for t in range(NT // 2):
                emit_v_tile(t)

            # Q/K weights after x/wv so the V pipeline starts sooner.
            nc.sync.dma_start(wq_sb[:], wq_d.rearrange("(o p) n -> p o n", p=P))
            nc.sync.dma_start(wk_sb[:], wk_d.rearrange("(o p) n -> p o n", p=P))

            wo_pool = ctx.enter_context(tc.tile_pool(name="wo", bufs=1))
            wo_sb = wo_pool.tile([P, NM, D], MD)
            nc.sync.dma_start(wo_sb[:], wo_d.rearrange("(f p) n -> p f n", p=P))

            # normalized O^T tiles until phase D; rows 0-63 = odd head (rr=1,
            # direct write), rows 64-127 = even head (rr=0, via DMA). wo rows
            # are swapped host-side to match.
            stg_pool = ctx.enter_context(tc.tile_pool(name="stg", bufs=1))
            stg_all = stg_pool.tile([P, NQC, NM, QW], MD)
            qkT_pool = ctx.enter_context(tc.tile_pool(name="qkT", bufs=2))
            pT_pool = ctx.enter_context(tc.tile_pool(name="pT", bufs=4))
            norm_pool = ctx.enter_context(tc.tile_pool(name="norm", bufs=2))
            sums_dram = ctx.enter_context(tc.tile_pool(name="sumsd", bufs=4, space="DRAM"))
            ostg_pool = ctx.enter_context(tc.tile_pool(name="ostg", bufs=3))

            def c1_emitters(hp):
                """Q^T/K^T projection for head-pair hp: per-head [64, S]
                tiles plus 8 emitter closures (one per psum accumulation
                group) for interleaved emission."""
                qT0 = qkT_pool.tile([HD, S], MD, tag="qT0", name="qT0")
                qT1 = qkT_pool.tile([HD, S], MD, tag="qT1", name="qT1")
                kT0 = qkT_pool.tile([HD, S], MD, tag="kT0", name="kT0")
                kT1 = qkT_pool.tile([HD, S], MD, tag="kT1", name="kT1")
                qTs = [qT0[:], qT1[:]]
                kTs = [kT0[:], kT1[:]]
                emitters = []
                for n in range(S // W5):
                    for w_sb, b_sb, dsts in ((wq_sb, bq_sb, qTs), (wk_sb, bk_sb, kTs)):
                        def emit(n=n, w_sb=w_sb, b_sb=b_sb, dsts=dsts):
                            sl = slice(n * W5, (n + 1) * W5)
                            psq = s_pool(slot[0]).tile(
                                [P, W5], F32, tag=f"s{slot[0] % 3}", name="psq"
                            )
                            slot[0] += 1
                            for d in range(ND):
                                nc.tensor.matmul(
                                    psq[:], w_sb[:, d, hp * P:(hp + 1) * P],
                                    xT[:, d, sl],
                                    start=(d == 0), stop=(d == ND - 1),
                                )
                            nc.vector.tensor_scalar_add(
                                dsts[0][:, sl], psq[0:HD, :], b_sb[0:HD, hp:hp + 1])
                            nc.vector.tensor_scalar_add(
                                dsts[1][:, sl], psq[HD:P, :], b_sb[HD:P, hp:hp + 1])
                        emitters.append(emit)
                return qTs, kTs, emitters

            d_done = set()
            d_fill = []   # phase-D pso closures, pumped as PE filler

            def queue_d_chunk(half, qc):
                """Partial output projection for q-chunk qc over head pairs
                f in {2*half, 2*half+1}: 16 independent pso closures."""
                if (half, qc) in d_done:
                    return
                d_done.add((half, qc))
                dst_d = out01_d if half == 0 else out23_d
                fs = (2 * half, 2 * half + 1)
                for mm in range(QW // P):
                    m = qc * (QW // P) + mm
                    off = mm * P
                    for n in range(D // WD):
                        def go(evac="v", m=m, off=off, n=n, qc=qc, fs=fs, dst_d=dst_d):
                            pso = s_pool(slot[0]).tile(
                                [P, WD], F32, tag=f"s{slot[0] % 3}", name="pso"
                            )
                            slot[0] += 1
                            for j, f in enumerate(fs):
                                nc.tensor.matmul(
                                    pso[:], stg_all[:, qc, f, off:off + P],
                                    wo_sb[:, f, n * WD:(n + 1) * WD],
                                    start=(j == 0), stop=(j == len(fs) - 1),
                                )
                            ost = ostg_pool.tile([P, WD], F32, tag="ostg", name="ost")
                            if evac == "s":
                                nc.scalar.copy(ost[:], pso[:])
                            else:
                                nc.vector.tensor_copy(ost[:], pso[:])
                            nc.sync.dma_start(
                                dst_d[m * P:(m + 1) * P, n * WD:(n + 1) * WD], ost[:]
                            )
                        d_fill.append(go)

            def norm_chunk(hp, rr, qc, o_acc):
                """Normalize finished O^T chunk: rows 0..HD-1 scaled by
                1/sums (row HD), written into stg. O^T and the sums row are
                evacuated to SBUF right away (frees the PSUM slot); the
                multiply runs on the otherwise-idle GpSimd engine. rr=1
                writes stg rows 0-63 directly; rr=0 goes via tmpn + a
                partition-shift DMA."""
                oc = norm_pool.tile([HD, QW], F32, tag="oc", name="oc")
                nc.vector.tensor_copy(oc[:], o_acc[0:HD, :])
                srow = norm_pool.tile([1, QW], F32, tag="srow", name="srow")
                nc.vector.tensor_copy(srow[:], o_acc[HD:VW, :])
                rd1 = sums_dram.tile([1, QW], F32, tag="rd1", name="rd1")
                rd1_ap = rd1[:]
                nc.sync.dma_start(rd1_ap, srow[:])
                sq = norm_pool.tile([P, SQF], F32, tag="sq", name="sq")
                nc.sync.dma_start(
                    sq[:],
                    bass.AP(tensor=rd1_ap.tensor, offset=rd1_ap.offset,
                            ap=[[SQF, P], [1, SQF]]),
                )
                nc.vector.reciprocal(sq[:], sq[:])
                rd2 = sums_dram.tile([1, QW], F32, tag="rd2", name="rd2")
                rd2_ap = rd2[:]
                nc.sync.dma_start(
                    bass.AP(tensor=rd2_ap.tensor, offset=rd2_ap.offset,
                            ap=[[SQF, P], [1, SQF]]),
                    sq[:],
                )
                bc = norm_pool.tile([HD, QW], F32, tag="bc", name="bc")
                nc.sync.dma_start(
                    bc[:],
                    bass.AP(tensor=rd2_ap.tensor, offset=rd2_ap.offset,
                            ap=[[0, HD], [1, QW]]),
                )
                if rr == 1:
                    nc.gpsimd.tensor_mul(
                        stg_all[0:HD, qc, hp, :], oc[:], bc[:]
                    )
                else:
                    tmpn = norm_pool.tile([HD, QW], MD, tag="tmpn", name="tmpn")
                    nc.gpsimd.tensor_mul(tmpn[:], oc[:], bc[:])
                    nc.sync.dma_start(stg_all[HD:P, qc, hp, :], tmpn[:])

            # ---- attention over head pairs -------------------------------
            cur = c1_emitters(0)
            for em in cur[2]:
                em()

            for hp in range(NM):
                qTs, kTs = cur[0], cur[1]
                if hp + 1 < NM:
                    nxt = c1_emitters(hp + 1)
                else:
                    nxt = None
                    xT_ctx.close()

                pend_pv = [None]  # delayed-by-one-step PV emitter

                def emit_scores(rr, ch, ki):
                    W0 = ch * SW
                    qlo = max(W0, ki * P)
                    rel = qlo - W0
                    s_ps = s_pool(slot[0]).tile(
                        [P, SW], F32, tag=f"s{slot[0] % 3}", name="s_ps"
                    )
                    slot[0] += 1
                    plo = rel
                    while plo < SW:
                        pw = min(512 - plo % 512, SW - plo)
                        nc.tensor.matmul(
                            s_ps[:, plo:plo + pw],
                            kTs[rr][:, ki * P:(ki + 1) * P],
                            qTs[rr][:, W0 + plo:W0 + plo + pw],
                            start=True, stop=True,
                        )
                        plo += pw
                    pT = pT_pool.tile([P, SW], MD, tag="pT", name="pT")
                    nc.scalar.activation(
                        pT[:, rel:SW], s_ps[:, rel:SW],
                        mybir.ActivationFunctionType.Exp, scale=SCALE,
                    )
                    if ki * P >= W0:
                        # diagonal tile: zero the strictly-lower part (on the
                        # otherwise-idle GpSimd engine, off the DVE queue)
                        nc.gpsimd.tensor_mul(
                            pT[:, rel:rel + P], pT[:, rel:rel + P], ut1[:]
                        )
                    return pT, rel

                def make_pv(rr, ch, ki, pT, rel, o_ps):
                    def emit_pv():
                        W0 = ch * SW
                        h = 2 * hp + rr
                        plo = rel
                        done = False
                        while plo < SW:
                            pw = min(512 - plo % 512, SW - plo)
                            lastki = (ch + 1) * (SW // P) - 1
                            if o_ps[rr][ch] is None:
                                o_ps[rr][ch] = ps_o.tile(
                                    [VW, QW], F32, tag="o", name="o_ps"
                                )
                            nc.tensor.matmul(
                                o_ps[rr][ch][:, plo:plo + pw],
                                v_sb[:, ki, h, :],
                                pT[:, plo:plo + pw],
                                start=(ki == 0), stop=(ki == lastki),
                            )
                            plo += pw
                            done = ki == lastki
                        if done:
                            norm_chunk(hp, rr, ch, o_ps[rr][ch])
                            o_ps[rr][ch] = None
                            if rr == 1 and hp == 1:
                                # out01 chunk ready (f0 from hp0, f1 just now)
                                if ch == 0:
                                    queue_d_chunk(0, 0)
                                # qc1 of out01 is deferred to hp3-ch0 (below)
                            if rr == 1 and hp == NM - 1:
                                queue_d_chunk(1, ch)
                    return emit_pv

                # fillers per window: V tiles 8-15 go into (hp0, ch0); the
                # next head-pair's QK projection spreads over both windows;
                # out01's deferred qc1 tops up hp2-ch1 and hp3-ch0.
                fill = {0: [], 1: []}
                if hp == 0:
                    fill[0] = [lambda t=t: emit_v_tile(t) for t in range(NT // 2, NT)]
                    if nxt is not None:
                        fill[1] = list(nxt[2])
                elif nxt is not None:
                    fill[0] = list(nxt[2][:3])
                    fill[1] = list(nxt[2][3:])
                if hp == 2:
                    fill[1] = fill[1] + [lambda: queue_d_chunk(0, 1)]

                o_ps = [[None] * NCH for _ in range(2)]
                for ch in range(NCH):
                    nki = ((ch + 1) * SW) // P
                    # window-sequential heads: all of rr=0's k-tiles, then
                    # rr=1's — keeps a single O^T accumulation live at a time
                    steps = [(rr, ki) for rr in (0, 1) for ki in range(nki)]
                    fillers = fill[ch]
                    emitted = 0
                    for i, (rr, ki) in enumerate(steps):
                        pT, rel = emit_scores(rr, ch, ki)
                        if pend_pv[0] is not None:
                            pend_pv[0]()
                        pend_pv[0] = make_pv(rr, ch, ki, pT, rel, o_ps)
                        want = ((i + 1) * len(fillers)) // len(steps)
                        while emitted < want:
                            fillers[emitted]()
                            emitted += 1
                        if d_fill and i % 3 == 0:
                            d_fill.pop(0)()
                # flush the delayed PV at the end of the head-pair
                if pend_pv[0] is not None:
                    pend_pv[0]()
                    pend_pv[0] = None
                if hp == 0:
                    wv_ctx.close()
                cur = nxt

            # ---- phase D remainder ---------------------------------------
            # attention is done here: alternate pso evacuation between DVE
            # and the now-idle ACT engine so the drain runs at PE pace.
            for half in range(2):
                for qc in range(NQC):
                    queue_d_chunk(half, qc)
            di = 0
            while d_fill:
                d_fill.pop(0)("s" if di % 2 == 0 else "v")
                di += 1

    nc.compile()
    return nc


_NC_CACHE = {}


def _get_nc(S, D, HN, HD):
    key = (S, D, HN, HD)
    if key not in _NC_CACHE:
        _NC_CACHE[key] = build_nc(S, D, HN, HD)
    return _NC_CACHE[key]


def augment_v(wv_local, bv_local, HN, HD):
    """Append per head a zero weight column with bias 1.0 (softmax-sum col)."""
    D = wv_local.shape[0]
    wv_a = np.zeros((D, HN, HD + 1), dtype=np.float32)
    wv_a[:, :, :HD] = wv_local.reshape(D, HN, HD)
    bv_a = np.ones((HN, HD + 1), dtype=np.float32)
    bv_a[:, :HD] = bv_local.reshape(HN, HD)
    return np.ascontiguousarray(wv_a.reshape(D, -1)), np.ascontiguousarray(bv_a.reshape(-1))


def kernel(**inputs):
    out, _ = run_with_results(inputs)
    return out


def run_with_results(inputs, **spmd_kwargs):
    from concourse.bass_utils import run_bass_kernel_spmd
    import ml_dtypes

    bf16 = ml_dtypes.bfloat16

    x = np.asarray(inputs["x"], dtype=np.float32)
    wq = np.asarray(inputs["wq"], dtype=np.float32)
    bq = np.asarray(inputs["bq"], dtype=np.float32)
    wk = np.asarray(inputs["wk"], dtype=np.float32)
    bk = np.asarray(inputs["bk"], dtype=np.float32)
    wv = np.asarray(inputs["wv"], dtype=np.float32)
    bv = np.asarray(inputs["bv"], dtype=np.float32)
    wo = np.asarray(inputs["wo"], dtype=np.float32)
    bo = np.asarray(inputs["bo"], dtype=np.float32)

    B, S, D = x.shape
    H = 16
    HD = D // H
    G = 2                  # head groups
    HN = H // G            # heads per core
    C = HN * HD
    n_cores = B * G

    nc = _get_nc(S, D, HN, HD)

    in_maps = []
    for c in range(n_cores):
        b, g = c // G, c % G
        sl = slice(g * C, (g + 1) * C)
        # stg packs the odd head in rows 0-63 and the even head in rows
        # 64-127 of each 128-row block; swap wo's rows to match.
        wo_loc = wo[sl, :].reshape(HN // 2, 2, HD, D)[:, ::-1]
        wo_loc = np.ascontiguousarray(wo_loc.reshape(C, D))
        wv_a, bv_a = augment_v(wv[:, sl], bv[sl], HN, HD)
        in_maps.append({
            "xT": np.ascontiguousarray(x[b].T).astype(bf16),
            "wq": np.ascontiguousarray(wq[:, sl]).astype(bf16),
            "wk": np.ascontiguousarray(wk[:, sl]).astype(bf16),
            "wv": wv_a.astype(bf16),
            "wo": wo_loc.astype(bf16),
            "bq": np.ascontiguousarray(bq[sl]),
            "bk": np.ascontiguousarray(bk[sl]),
            "bv": bv_a,
        })

    res = run_bass_kernel_spmd(nc, in_maps, core_ids=list(range(n_cores)), **spmd_kwargs)
    outs = [m["out01"] + m["out23"] for m in res.results]
    out = np.stack([sum(outs[b * G + g] for g in range(G)) for b in range(B)])
    out = out + bo[None, None, :]
    return out.astype(np.float32), res
